# revision 10
# baseline (speedup 1.0000x reference)
"""4-layer GAT (PyG GATConv semantics) on 8 Trainium2 NeuronCores via Bass/Tile.

Sharding: nodes are split into 8 contiguous ranges (dst-partitioning); each
core owns the edges whose dst lands in its range (graph partitioning done on
host).  Per layer:
  1. dense phase (per core, own nodes): xp = h @ W' (BN scale folded),
     es/ed head scores via host-precombined [din, H] matrices; rows
     [xp(bf16) || es(f32) || pad] written to a DRAM table slice.  es/ed/xp
     are also kept in SBUF for the local self-loop term.
  2. Chunked AllGather (4 slices, fired as dense groups complete) so every
     core sees all N rows; table rows are slice-major interleaved and the
     host remaps gather indices accordingly.
  3. edge phase: per 128-node dst group, bulk dma_gather of src rows (int16
     indices, -1 padding so the Q7 ucode trims trailing pads; src split into
     two 32K halves), per-edge softmax numerators z = exp(leaky_relu(es+ed))
     on the scalar engine (ed delivered by an indicator matmul from the
     core-local table), and a 0/1-indicator bf16 matmul (indicator built
     on-chip via is_equal against an iota tile) that segment-sums z*xp and z
     into PSUM; self-loops are handled locally (identity matmul), not
     gathered.  Divide by the z-sum, add folded BN shift, ReLU.
Final: graph mean-pool via the same indicator-matmul trick, AllReduce the
[64, HC] partials, then the tiny MLP + sigmoid on every core.
"""

import ml_dtypes
import numpy as np

P = 128
N_GRAPHS = 64
NEG_SLOPE = 0.2
BN_EPS = 1e-5
NC_BLK = int(__import__("os").environ.get("GAT_BLK", "8"))
HALF = 32768     # int16 index range per table half
import os as _os0
AG_SLICES = int(_os0.environ.get("GAT_AGS", "4"))

# full-size problem constants (hardcoded per harness contract)
FULL = dict(N=50000, FIN=128, H=4, C=64, n_cores=8)


def _ceil(a, b):
    return -(-a // b)


# ----------------------------------------------------------------------------
# host-side preprocessing
# ----------------------------------------------------------------------------

def preprocess(inputs, N, FIN, H, C, n_cores):
    """Partition graph + fold BN + build per-core device input maps."""
    HC = H * C
    ROWP = 512   # fp8 elems (bytes) per table row: xp fp8 | es f32 | pad
    npc = N // n_cores
    ngrp = _ceil(npc, P)

    # allgather slice layout (group-aligned): slice s covers local rows
    # [sl_r0[s], sl_r1[s]); table row order is slice-major, core-major.
    gsl = ([13, 13, 13, ngrp - 39] if (AG_SLICES == 4 and ngrp >= 40)
           else [ngrp] + [0] * (AG_SLICES - 1))
    sl_r0, sl_sz, base = [], [], []
    acc_r, acc_b = 0, 0
    for s in range(AG_SLICES):
        r0 = acc_r
        r1 = min(npc, r0 + gsl[s] * P)
        sl_r0.append(r0)
        sl_sz.append(r1 - r0)
        base.append(acc_b)
        acc_r = r1
        acc_b += n_cores * (r1 - r0)
    assert acc_r == npc and acc_b == N

    def table_row(src):
        k = src // npc
        r = src % npc
        s = np.searchsorted(np.array(sl_r0 + [npc]), r, side="right") - 1
        return base[s] + k * sl_sz[s] + (r - sl_r0[s])

    # vectorized table_row
    src_all = np.arange(N, dtype=np.int64)
    k_all = src_all // npc
    r_all = src_all % npc
    s_all = np.searchsorted(np.array(sl_r0 + [npc]), r_all, side="right") - 1
    base_a = np.array(base)[s_all]
    sz_a = np.array(sl_sz)[s_all]
    r0_a = np.array(sl_r0)[s_all]
    perm = base_a + k_all * sz_a + (r_all - r0_a)   # node id -> table row

    ei = np.asarray(inputs["edge_index"])
    src = ei[0].astype(np.int64)
    dst = ei[1].astype(np.int64)
    # self-loops handled locally; drop any real (i, i) edges? PyG GATConv
    # adds self-loops on top of existing edges, so (i, i) real edges stay.

    order = np.argsort(dst, kind="stable")
    src_s, dst_s = src[order], dst[order]
    src_t = perm[src_s]                      # permuted table rows
    core_of = dst_s // npc
    loc = dst_s - core_of * npc
    grp_of = loc // P
    is_high = src_t >= HALF

    # per (core, group): low-src edges then high-src edges, each padded to
    # a multiple of 128; chunk counts maxed across cores (SPMD program).
    cntl = np.zeros((n_cores, ngrp), dtype=np.int64)
    cnth = np.zeros((n_cores, ngrp), dtype=np.int64)
    np.add.at(cntl, (core_of[~is_high], grp_of[~is_high]), 1)
    np.add.at(cnth, (core_of[is_high], grp_of[is_high]), 1)
    nchl = np.maximum(1, _ceil(cntl, P).max(axis=0))
    nchh = _ceil(cnth, P).max(axis=0)
    if N <= HALF:
        nchh[:] = 0
    nch = nchl + nchh
    chunk_base = np.concatenate([[0], np.cumsum(nch)]).astype(np.int64)
    TC = int(chunk_base[-1])

    import os as _os1
    _pad = -1 if _os1.environ.get("GAT_NEG", "0") == "1" else 0
    src16 = np.full((n_cores, 16, TC * 8), _pad, dtype=np.int16)
    dstl_arr = np.full((n_cores, P, TC), -1.0, dtype=np.float32)

    def place(k, g, edges_src, edges_dst, base_chunk, high):
        """edges sorted by dst; slot j -> (p=j%128, chunk=base+j//128)."""
        n_e = edges_src.shape[0]
        if n_e == 0:
            return
        j = np.arange(n_e)
        p_idx = j % P
        c_idx = base_chunk + j // P
        v = edges_src - (HALF if high else 0)
        src16[k, p_idx % 16, c_idx * 8 + p_idx // 16] = v.astype(np.int16)
        dstl_arr[k, p_idx, c_idx] = edges_dst.astype(np.float32)

    for k in range(n_cores):
        m_core = core_of == k
        sk, dk, gk, hk = (src_t[m_core], loc[m_core], grp_of[m_core],
                          is_high[m_core])
        for g in range(ngrp):
            m = gk == g
            sg, dg, hg = sk[m], dk[m] - g * P, hk[m]
            cb = int(chunk_base[g])
            place(k, g, sg[~hg], dg[~hg], cb, False)
            if nchh[g]:
                place(k, g, sg[hg], dg[hg], cb + int(nchl[g]), True)
    src16 = np.tile(src16, (1, 8, 1))   # replicate across 16-partition groups

    # indicator matrix M[p=node, c, e] for ed delivery, shipped bf16
    # (edge (p, chunk c) has dst node dstl_arr[p, c]); the edge->node
    # indicator (scatter lhsT) is built on-chip from dstl via is_equal.
    nrng = np.arange(P, dtype=np.float32)
    use_eq = __import__("os").environ.get("GAT_EQ", "1") == "1"
    m_ship = np.zeros((n_cores, P, TC * P), dtype=ml_dtypes.bfloat16)
    mt_ship = (None if use_eq else
               np.zeros((n_cores, P, TC * P), dtype=ml_dtypes.bfloat16))
    for k in range(n_cores):
        eq = (dstl_arr[k][:, :, None] == nrng[None, None, :])  # [e, c, n]
        m_ship[k] = np.ascontiguousarray(
            eq.transpose(2, 1, 0)).astype(ml_dtypes.bfloat16).reshape(
                P, TC * P)
        if not use_eq:
            mt_ship[k] = eq.astype(ml_dtypes.bfloat16).reshape(P, TC * P)

    # pooling batch ids per (p, g); -1 for pad nodes
    batch = np.asarray(inputs["batch"]).astype(np.int64)
    bat_arr = np.full((n_cores, P, ngrp), -1.0, dtype=np.float32)
    for k in range(n_cores):
        bk = batch[k * npc:(k + 1) * npc]
        for g in range(ngrp):
            rows = bk[g * P:(g + 1) * P]
            bat_arr[k, :rows.shape[0], g] = rows.astype(np.float32)

    # fold BN into weights
    wcats, treps = [], []
    dins = [FIN, HC, HC, HC]
    for l in range(1, 5):
        W = np.asarray(inputs[f"W{l}"], np.float32)
        a_s = np.asarray(inputs[f"as{l}"], np.float32)
        a_d = np.asarray(inputs[f"ad{l}"], np.float32)
        b = np.asarray(inputs[f"b{l}"], np.float32)
        g_ = np.asarray(inputs[f"g{l}"], np.float32)
        be = np.asarray(inputs[f"be{l}"], np.float32)
        rm = np.asarray(inputs[f"rm{l}"], np.float32)
        rv = np.asarray(inputs[f"rv{l}"], np.float32)
        S = g_ / np.sqrt(rv + BN_EPS)
        T = (b - rm) * S + be
        Wp = W * S[None, :]
        Wr = W.reshape(dins[l - 1], H, C)
        Aes = np.einsum("dhc,hc->dh", Wr, a_s).astype(np.float32)
        Aed = np.einsum("dhc,hc->dh", Wr, a_d).astype(np.float32)
        wcats.append(np.concatenate([Wp, Aes, Aed], axis=1).astype(np.float32))
        treps.append(np.tile(T[None, :], (P, 1)).astype(np.float32))

    x = np.asarray(inputs["x"], np.float32)
    iota = np.tile(np.arange(P, dtype=np.float32)[None, :], (P, 1))
    ident = np.eye(P, dtype=np.float32)
    identb = np.eye(P, dtype=ml_dtypes.bfloat16)
    cntf = np.bincount(batch, minlength=N_GRAPHS).astype(np.float32)
    rcinv = (1.0 / np.clip(cntf, 1.0, None)).reshape(N_GRAPHS, 1).astype(
        np.float32)
    wm1 = np.asarray(inputs["Wm1"], np.float32)
    bm1 = np.tile(np.asarray(inputs["bm1"], np.float32)[None, :],
                  (N_GRAPHS, 1))
    wm2 = np.asarray(inputs["Wm2"], np.float32)
    bm2 = np.tile(np.asarray(inputs["bm2"], np.float32)[None, :],
                  (N_GRAPHS, 1))

    meta = dict(
        N=N, FIN=FIN, H=H, C=C, HC=HC, ROWP=ROWP, npc=npc, ngrp=ngrp,
        nchl=[int(v) for v in nchl], nchh=[int(v) for v in nchh],
        chunk_base=[int(v) for v in chunk_base], TC=TC, n_cores=n_cores,
        gsl=gsl, sl_r0=sl_r0, sl_sz=sl_sz, sl_base=base,
    )

    in_maps = []
    for k in range(n_cores):
        m = dict(
            x_own=np.ascontiguousarray(x[k * npc:(k + 1) * npc]),
            src16=np.ascontiguousarray(src16[k]),
            m_ship=m_ship[k],
            dstl=np.ascontiguousarray(dstl_arr[k]),
            **({} if use_eq else {"mt_ship": mt_ship[k]}),
            batchf=np.ascontiguousarray(bat_arr[k]),
            iota=iota, ident=ident, identb=identb, rcinv=rcinv,
            wm1=wm1, bm1=bm1, wm2=wm2, bm2=bm2,
        )
        for l in range(1, 5):
            m[f"wcat{l}"] = wcats[l - 1]
            m[f"trep{l}"] = treps[l - 1]
        in_maps.append(m)
    return meta, in_maps


# ----------------------------------------------------------------------------
# bass program
# ----------------------------------------------------------------------------

def build_bass(meta):
    import concourse.bacc as bacc
    import concourse.bass as bass
    import concourse.mybir as mybir
    import concourse.tile as tile
    from contextlib import ExitStack

    f32 = mybir.dt.float32
    f32r = mybir.dt.float32r
    bf16 = mybir.dt.bfloat16
    i16 = mybir.dt.int16
    f8 = mybir.dt.float8e4
    Alu = mybir.AluOpType
    Act = mybir.ActivationFunctionType

    N, FIN, H, C, HC = meta["N"], meta["FIN"], meta["H"], meta["C"], meta["HC"]
    ROWP, npc, ngrp, TC = meta["ROWP"], meta["npc"], meta["ngrp"], meta["TC"]
    nchl, nchh, chunk_base = meta["nchl"], meta["nchh"], meta["chunk_base"]
    n_cores = meta["n_cores"]
    sl_r0, sl_sz, sl_base = meta["sl_r0"], meta["sl_sz"], meta["sl_base"]
    gsl = meta["gsl"]
    DROW = HC + 2 * H         # dense-phase psum row (f32)
    VW = HC + H               # scatter rhs row: values | z
    dins = [FIN, HC, HC, HC]
    RG = [list(range(n_cores))]

    nc = bacc.Bacc("TRN2", target_bir_lowering=False, debug=False,
                   num_devices=n_cores, num_swdge_queues=4)

    # I/O
    t_x = nc.dram_tensor("x_own", [npc, FIN], f32, kind="ExternalInput")
    t_s16 = nc.dram_tensor("src16", [P, TC * 8], i16, kind="ExternalInput")
    t_m = nc.dram_tensor("m_ship", [P, TC * P], bf16, kind="ExternalInput")
    use_eq = __import__("os").environ.get("GAT_EQ", "1") == "1"
    t_mt = (None if use_eq else
            nc.dram_tensor("mt_ship", [P, TC * P], bf16,
                           kind="ExternalInput"))
    t_dstl = nc.dram_tensor("dstl", [P, TC], f32, kind="ExternalInput")
    t_bat = nc.dram_tensor("batchf", [P, ngrp], f32, kind="ExternalInput")
    t_iota = nc.dram_tensor("iota", [P, P], f32, kind="ExternalInput")
    t_ident = nc.dram_tensor("ident", [P, P], f32, kind="ExternalInput")
    t_identb = nc.dram_tensor("identb", [P, P], bf16, kind="ExternalInput")
    t_rcinv = nc.dram_tensor("rcinv", [N_GRAPHS, 1], f32, kind="ExternalInput")
    t_wcat = [nc.dram_tensor(f"wcat{l}", [dins[l - 1], DROW], f32,
                             kind="ExternalInput") for l in range(1, 5)]
    t_trep = [nc.dram_tensor(f"trep{l}", [P, HC], f32, kind="ExternalInput")
              for l in range(1, 5)]
    t_wm1 = nc.dram_tensor("wm1", [HC, 32], f32, kind="ExternalInput")
    t_bm1 = nc.dram_tensor("bm1", [N_GRAPHS, 32], f32, kind="ExternalInput")
    t_wm2 = nc.dram_tensor("wm2", [32, 1], f32, kind="ExternalInput")
    t_bm2 = nc.dram_tensor("bm2", [N_GRAPHS, 1], f32, kind="ExternalInput")
    t_out = nc.dram_tensor("out", [N_GRAPHS, 1], f32, kind="ExternalOutput")

    cc_in = [nc.dram_tensor(f"cc_in{l}", [npc, ROWP], f8)
             for l in range(1, 5)]
    table = [nc.dram_tensor(f"table{l}", [N, ROWP], f8, addr_space="Shared")
             for l in range(1, 5)]
    ar_in = nc.dram_tensor("ar_in", [N_GRAPHS, HC], f32)
    ar_out = nc.dram_tensor("ar_out", [N_GRAPHS, HC], f32,
                            addr_space="Shared")

    with tile.TileContext(nc) as tc, ExitStack() as ctx:
        cpool = ctx.enter_context(tc.tile_pool(name="consts", bufs=1))
        wpool = ctx.enter_context(tc.tile_pool(name="weights", bufs=1))
        work = ctx.enter_context(tc.tile_pool(name="work", bufs=3))
        gpool = ctx.enter_context(tc.tile_pool(name="gath", bufs=4))
        psum = ctx.enter_context(tc.tile_pool(name="psum", bufs=2,
                                              space="PSUM"))
        gps_pool = ctx.enter_context(tc.tile_pool(name="gpsum", bufs=2,
                                                  space="PSUM"))

        # ---- constants ----
        iota_t = cpool.tile([P, P], f32, tag="iota")
        ident_t = cpool.tile([P, P], f32, tag="ident")
        identb_t = cpool.tile([P, P], bf16, tag="identb")
        s16_t = cpool.tile([P, TC * 8], i16, tag="s16")
        dstl_t = cpool.tile([P, TC], f32, tag="dstl")
        bat_t = cpool.tile([P, ngrp], f32, tag="bat")
        for tt, src_dram in ((iota_t, t_iota), (ident_t, t_ident),
                             (identb_t, t_identb), (s16_t, t_s16),
                             (dstl_t, t_dstl), (bat_t, t_bat)):
            nc.sync.dma_start(out=tt[:], in_=src_dram[:])

        wcat_t = []
        for l in range(4):
            din = dins[l]
            ks_t = []
            for ks in range(_ceil(din, P)):
                ksz = min(P, din - ks * P)
                wt = wpool.tile([ksz, DROW], f32, tag=f"wcat{l}_{ks}")
                nc.sync.dma_start(out=wt[:],
                                  in_=t_wcat[l][ks * P:ks * P + ksz, :])
                ks_t.append(wt)
            wcat_t.append(ks_t)
        trep_t = []
        for l in range(4):
            tt = wpool.tile([P, HC], f32, tag=f"trep{l}")
            nc.sync.dma_start(out=tt[:], in_=t_trep[l][:])
            trep_t.append(tt)
        wm1_t = []
        for ks in range(_ceil(HC, P)):
            ksz = min(P, HC - ks * P)
            wt = wpool.tile([ksz, 32], f32, tag=f"wm1_{ks}")
            nc.sync.dma_start(out=wt[:], in_=t_wm1[ks * P:ks * P + ksz, :])
            wm1_t.append(wt)
        rcinv_t = wpool.tile([N_GRAPHS, 1], f32, tag="rcinv")
        bm1_t = wpool.tile([N_GRAPHS, 32], f32, tag="bm1")
        wm2_t = wpool.tile([32, 1], f32, tag="wm2")
        bm2_t = wpool.tile([N_GRAPHS, 1], f32, tag="bm2")
        nc.sync.dma_start(out=rcinv_t[:], in_=t_rcinv[:])
        nc.sync.dma_start(out=bm1_t[:], in_=t_bm1[:])
        nc.sync.dma_start(out=wm2_t[:], in_=t_wm2[:])
        nc.sync.dma_start(out=bm2_t[:], in_=t_bm2[:])

        pool_in = cpool.tile([P, ngrp, HC], bf16, tag="poolin")
        ed_own = cpool.tile([P, ngrp, H], bf16, tag="edown")
        es_own = cpool.tile([P, ngrp, H], bf16, tag="esown")
        xp_own = cpool.tile([P, ngrp, HC], f8, tag="xpown")
        gt_init = [0]   # gather tiles needing one-time memset
        negb = cpool.tile([P, 1], f32, tag="negb")
        nc.vector.memset(negb[:], -5.0)
        import os as _os
        use_prep = _os.environ.get("GAT_PREP", "0") == "1"
        dma_sems = ([nc.alloc_semaphore("gatdma0"), nc.alloc_semaphore("gatdma1")]
                    if use_prep else None)

        def dense(l, g, h_ap):
            """layer-(l+1) dense phase for group g; h_ap [P, din] f32 sbuf."""
            din = dins[l]
            nks = _ceil(din, P)
            gpn = min(npc - g * P, P)
            hT = []
            for ks in range(nks):
                ksz = min(P, din - ks * P)
                tp = psum.tile([P, P], f32, tag="transp")
                nc.tensor.transpose(tp[:ksz, :], h_ap[:, ks * P:ks * P + ksz],
                                    ident_t[:])
                ht = work.tile([P, P], f32, tag=f"hT{ks}")
                nc.vector.tensor_copy(ht[:ksz, :], tp[:ksz, :])
                hT.append(ht)
            dp = psum.tile([P, DROW], f32, tag="dens")
            for ks in range(nks):
                ksz = min(P, din - ks * P)
                nc.tensor.matmul(dp[:], lhsT=hT[ks][:ksz, :],
                                 rhs=wcat_t[l][ks][:ksz, :],
                                 start=(ks == 0), stop=(ks == nks - 1))
            nc.vector.tensor_copy(es_own[:, g, :], dp[:, HC:HC + H])
            nc.vector.tensor_copy(ed_own[:, g, :], dp[:, HC + H:HC + 2 * H])
            st = work.tile([P, ROWP], f8, tag="stage")
            nc.vector.tensor_tensor(out=st[:, 0:HC], in0=dp[:, 0:HC],
                                    in1=trep_t[l][:], op=Alu.add)
            nc.vector.tensor_copy(st[:, HC:HC + 16].bitcast(f32),
                                  dp[:, HC:HC + H])
            nc.scalar.activation(xp_own[:, g, :], st[:, 0:HC], Act.Copy)
            nc.sync.dma_start(out=cc_in[l][g * P:g * P + gpn, 0:HC + 16],
                              in_=st[:gpn, 0:HC + 16])

        def edge(l, g):
            """edge phase of layer l (1-based) for dst group g."""
            n_c = nchl[g] + nchh[g]
            cb = chunk_base[g]
            gps = gps_pool.tile([P, VW], f32, tag="grp")

            # local self-loop term: z_s = exp(lrelu(es+ed)), identity scatter
            zsl = work.tile([P, H], f32, tag="zself")
            nc.vector.tensor_tensor(out=zsl[:], in0=es_own[:, g, :],
                                    in1=ed_own[:, g, :], op=Alu.add)
            zsl2 = work.tile([P, H], f32, tag="zself2")
            nc.vector.scalar_tensor_tensor(
                out=zsl2[:], in0=zsl[:], scalar=NEG_SLOPE, in1=zsl[:],
                op0=Alu.mult, op1=Alu.max)
            zslf = work.tile([P, H], f8, tag="zselff")
            nc.scalar.activation(zslf[:], zsl2[:], Act.Exp, bias=negb[:])
            vs = gpool.tile([P, VW], bf16, tag="vself")
            nc.scalar.activation(vs[:, HC:HC + H], zslf[:], Act.Copy)
            nc.vector.tensor_tensor(
                out=vs[:, 0:HC].rearrange("p (h c) -> p h c", c=C),
                in0=xp_own[:, g, :].rearrange("p (h c) -> p h c", c=C),
                in1=zslf[:].unsqueeze(2).to_broadcast([P, H, C]),
                op=Alu.mult)
            nc.tensor.matmul(gps[:], lhsT=identb_t[:], rhs=vs[:],
                             start=True, stop=False)

            blocks = []
            for b0 in range(0, nchl[g], NC_BLK):
                blocks.append((b0, min(NC_BLK, nchl[g] - b0), False))
            for b0 in range(nchl[g], n_c, NC_BLK):
                blocks.append((b0, min(NC_BLK, n_c - b0), True))
            for b0, nbc, high in blocks:
                c0 = cb + b0
                gt = gpool.tile([P, NC_BLK, ROWP], f8, tag="gath")
                if gt_init[0] < 3:
                    nc.vector.memset(gt[:], 0.0)
                    gt_init[0] += 1
                in_ap = table[l - 1][HALF:, :] if high else table[l - 1][:]
                qn = (1 if high else 0) if __import__("os").environ.get("GAT_2Q", "1") == "1" else 0
                if use_prep:
                    nc.gpsimd.dma_gather(
                        out_ap=gt[:, :nbc, :], in_ap=in_ap,
                        idxs_ap=s16_t[:, c0 * 8:(c0 + nbc) * 8],
                        num_idxs=nbc * P, num_idxs_reg=nbc * P,
                        elem_size=ROWP, queue_num=qn,
                        prepare_only=True, sem=dma_sems[qn],
                    )
                    nc.gpsimd.trigger_dma(count=None, queue_num=qn)
                else:
                    nc.gpsimd.dma_gather(
                        out_ap=gt[:, :nbc, :], in_ap=in_ap,
                        idxs_ap=s16_t[:, c0 * 8:(c0 + nbc) * 8],
                        num_idxs=nbc * P, num_idxs_reg=nbc * P,
                        elem_size=ROWP, queue_num=qn,
                    )
                # on-chip edge->node indicator (scatter lhsT)
                mtt = gpool.tile([P, NC_BLK, P], bf16, tag="mt")
                if use_eq:
                    nc.vector.tensor_tensor(
                        out=mtt[:, :nbc, :],
                        in0=dstl_t[:, c0:c0 + nbc].unsqueeze(2).to_broadcast(
                            [P, nbc, P]),
                        in1=iota_t[:, 0:P].unsqueeze(1).to_broadcast(
                            [P, nbc, P]),
                        op=Alu.is_equal)
                else:
                    nc.sync.dma_start(out=mtt[:, :nbc, :],
                                      in_=t_mt[:, c0 * P:(c0 + nbc) * P])
                mm = gpool.tile([P, NC_BLK, P], bf16, tag="mm")
                nc.sync.dma_start(out=mm[:, :nbc, :],
                                  in_=t_m[:, c0 * P:(c0 + nbc) * P])
                edp = psum.tile([P, NC_BLK * H], f32, tag="edp")
                for c in range(nbc):
                    nc.tensor.matmul(edp[:, c * H:(c + 1) * H],
                                     lhsT=mm[:, c, :], rhs=ed_own[:, g, :],
                                     start=True, stop=True)
                sc = work.tile([P, NC_BLK, H], f32, tag="sc")
                es_ap = gt[:, :nbc, HC:HC + 16].bitcast(f32)
                nc.vector.tensor_tensor(
                    out=sc[:, :nbc, :], in0=es_ap,
                    in1=edp[:, 0:nbc * H].rearrange("p (n h) -> p n h", h=H),
                    op=Alu.add)
                sc2 = work.tile([P, NC_BLK, H], f32, tag="sc2")
                nc.vector.scalar_tensor_tensor(
                    out=sc2[:, :nbc, :], in0=sc[:, :nbc, :],
                    scalar=NEG_SLOPE, in1=sc[:, :nbc, :],
                    op0=Alu.mult, op1=Alu.max)
                vt = gpool.tile([P, NC_BLK, VW], bf16, tag="vt")
                zst = work.tile([P, NC_BLK, H], f8, tag="zstf")
                nc.scalar.activation(zst[:, :nbc, :], sc2[:, :nbc, :],
                                     Act.Exp, bias=negb[:])
                nc.scalar.activation(vt[:, :nbc, HC:HC + H],
                                     zst[:, :nbc, :], Act.Copy)
                nc.vector.tensor_tensor(
                    out=vt[:, :nbc, 0:HC].rearrange(
                        "p n (h c) -> p n h c", c=C),
                    in0=gt[:, :nbc, 0:HC].rearrange(
                        "p n (h c) -> p n h c", c=C),
                    in1=zst[:, :nbc, :].unsqueeze(3).to_broadcast(
                        [P, nbc, H, C]),
                    op=Alu.mult,
                )
                for c in range(nbc):
                    nc.tensor.matmul(
                        gps[:], lhsT=mtt[:, c, :], rhs=vt[:, c, :],
                        start=False, stop=(b0 + c == n_c - 1),
                    )
            # postprocess: divide by z-sum, add BN shift, relu (layers 1-3)
            d4 = work.tile([P, H], f32, tag="d4")
            nc.vector.tensor_scalar_max(d4[:], gps[:, HC:HC + H], 1e-30)
            r4 = work.tile([P, H], f32, tag="r4")
            nc.vector.reciprocal(r4[:], d4[:])
            if l < 4:
                hr = work.tile([P, HC], f32, tag="hrelu")
                for h in range(H):
                    nc.scalar.activation(hr[:, h * C:(h + 1) * C],
                                         gps[:, h * C:(h + 1) * C],
                                         Act.Relu, scale=r4[:, h:h + 1])
                dense(l, g, hr[:])
            else:
                for h in range(H):
                    nc.scalar.activation(pool_in[:, g, h * C:(h + 1) * C],
                                         gps[:, h * C:(h + 1) * C],
                                         Act.Copy, scale=r4[:, h:h + 1])

        def allgather(l, s):
            r0, sz, b = sl_r0[s], sl_sz[s], sl_base[s]
            if sz == 0:
                return
            nc.gpsimd.collective_compute(
                "AllGather", Alu.bypass, replica_groups=RG,
                ins=[cc_in[l][r0:r0 + sz, :].opt()],
                outs=[table[l][b:b + n_cores * sz, :].opt()],
            )

        # group index after which AG slice s fires (last group of slice s)
        ag_after = []
        acc = 0
        for s in range(AG_SLICES):
            acc += gsl[s]
            ag_after.append(acc - 1)

        # ---- program ----
        for g in range(ngrp):
            gpn = min(npc - g * P, P)
            xg = work.tile([P, FIN], f32, tag="xg")
            if gpn < P:
                nc.vector.memset(xg[:], 0.0)
            nc.sync.dma_start(out=xg[:gpn, :], in_=t_x[g * P:g * P + gpn, :])
            dense(0, g, xg[:])
            for s in range(AG_SLICES):
                if g == ag_after[s]:
                    allgather(0, s)
        for l in range(1, 5):
            for g in range(ngrp):
                edge(l, g)
                if l < 4:
                    for s in range(AG_SLICES):
                        if g == ag_after[s]:
                            allgather(l, s)

        # ---- pooling ----
        pps = gps_pool.tile([N_GRAPHS, HC], f32, tag="grp")
        for g in range(ngrp):
            mb = work.tile([P, N_GRAPHS], bf16, tag="mb")
            nc.vector.tensor_tensor(
                out=mb[:],
                in0=bat_t[:, g:g + 1].to_broadcast([P, N_GRAPHS]),
                in1=iota_t[:, 0:N_GRAPHS],
                op=Alu.is_equal,
            )
            nc.tensor.matmul(pps[:], lhsT=mb[:], rhs=pool_in[:, g, :],
                             start=(g == 0), stop=(g == ngrp - 1))
        psb = work.tile([N_GRAPHS, HC], f32, tag="psb")
        nc.vector.tensor_copy(psb[:], pps[:])
        nc.sync.dma_start(out=ar_in[:], in_=psb[:])
        nc.gpsimd.collective_compute(
            "AllReduce", Alu.add, replica_groups=RG,
            ins=[ar_in[:].opt()], outs=[ar_out[:].opt()],
        )
        ps2 = work.tile([N_GRAPHS, HC], f32, tag="ps2")
        nc.sync.dma_start(out=ps2[:], in_=ar_out[:])
        hg = work.tile([N_GRAPHS, HC], f32, tag="hg")
        nc.vector.tensor_scalar_mul(hg[:], ps2[:, 0:HC], rcinv_t[:, 0:1])
        # MLP layer 1
        nks = _ceil(HC, P)
        z1p = psum.tile([N_GRAPHS, 32], f32, tag="dens")
        hgT = []
        for ks in range(nks):
            ksz = min(P, HC - ks * P)
            tp = psum.tile([P, N_GRAPHS], f32, tag="transp")
            nc.tensor.transpose(tp[:ksz, :], hg[:, ks * P:ks * P + ksz],
                                ident_t[:N_GRAPHS, :N_GRAPHS])
            ht = work.tile([P, N_GRAPHS], f32, tag=f"hgT{ks}")
            nc.vector.tensor_copy(ht[:ksz, :], tp[:ksz, :])
            hgT.append(ht)
        for ks in range(nks):
            ksz = min(P, HC - ks * P)
            nc.tensor.matmul(z1p[:], lhsT=hgT[ks][:ksz, :], rhs=wm1_t[ks][:],
                             start=(ks == 0), stop=(ks == nks - 1))
        z1 = work.tile([N_GRAPHS, 32], f32, tag="z1s")
        nc.vector.tensor_tensor(out=z1[:], in0=z1p[:], in1=bm1_t[:],
                                op=Alu.add)
        nc.vector.tensor_scalar_max(z1[:], z1[:], 0.0)
        # MLP layer 2
        tp2 = psum.tile([32, N_GRAPHS], f32, tag="transp")
        nc.tensor.transpose(tp2[:], z1[:], ident_t[:N_GRAPHS, :N_GRAPHS])
        z1T = work.tile([32, N_GRAPHS], f32, tag="z1Ts")
        nc.vector.tensor_copy(z1T[:], tp2[:])
        z2p = psum.tile([N_GRAPHS, 1], f32, tag="dens")
        nc.tensor.matmul(z2p[:], lhsT=z1T[:], rhs=wm2_t[:], start=True,
                         stop=True)
        ob = work.tile([N_GRAPHS, 1], f32, tag="ob")
        nc.vector.tensor_tensor(out=ob[:], in0=z2p[:], in1=bm2_t[:],
                                op=Alu.add)
        nc.scalar.activation(ob[:], ob[:], Act.Sigmoid)
        nc.sync.dma_start(out=t_out[:], in_=ob[:])

    nc.compile()
    return nc


# ----------------------------------------------------------------------------
# entry point
# ----------------------------------------------------------------------------

def kernel(**inputs):
    import concourse.bass_utils as bass_utils

    cfg = FULL
    meta, in_maps = preprocess(inputs, cfg["N"], cfg["FIN"], cfg["H"],
                               cfg["C"], cfg["n_cores"])
    nc = build_bass(meta)
    res = bass_utils.run_bass_kernel_spmd(
        nc, in_maps, core_ids=list(range(cfg["n_cores"])))
    return np.asarray(res.results[0]["out"], dtype=np.float32)
